# revision 15
# baseline (speedup 1.0000x reference)
"""Causal multi-head attention (B=4, H=16, S=2048, D=64) on 8 TRN2 NeuronCores.

Sharding: B*H = 64 (batch, head) pairs -> 8 per core, fully independent.

Per-core algorithm (per head), v2:
  - Q, K cast f32->bf16 into DRAM scratch [S, 64], then DMA-transposed
    TWICE into [128, S] SBUF tiles with duplicated halves (rows 0-63 and
    64-127 both hold the [64, S] transposed operand).
  - Scores S^T[kb] = K[kb] @ Q^T computed in row-tiled PAIRS: two
    concurrent 64-contraction matmuls on PE row groups 0-1 / 2-3, filling
    a [128, 1024] PSUM chunk ([0:512] + [512:CL]) -> ~2x QK^T throughput.
  - exp split across engines: most chunks ScalarE Exp activation
    (bf16 out); kb>=8 chunks + filler go to DVE as a one-instruction
    bf16-space Schraudolph: i16 = round(A16*x + B16) bitcast to bf16
    (rel err ~2%, capped share). Diagonal masking for the DVE chunks is
    folded in via a subtract-mask (saturation -> -0.0 -> zero prob);
    kb<8 diagonals use a bf16 triangular-mask multiply.
  - PV: O[qb] = sum_kb U^T[kb].T @ [V[kb] | 1] accumulated in one of 7
    65-column slots of a shared [128, 455] PSUM tile; per 7-slot group
    one strided reciprocal + one broadcast multiply normalizes into the
    f32 output tile.
"""

import numpy as np

import concourse.bass as bass
import concourse.tile as tile
from concourse import mybir
from concourse.bass_utils import run_bass_kernel_spmd
from concourse.masks import make_upper_triangular
from concourse.vector_clock import ScopedClock, VectorClock

F32 = mybir.dt.float32
BF16 = mybir.dt.bfloat16
I16 = mybir.dt.int16

B, H, S, D = 4, 16, 2048, 64
N_CORES = 8
HEADS_PER_CORE = B * H // N_CORES  # 8
NB = S // 128  # 16 blocks of 128
SCALE = 1.0 / np.sqrt(np.float32(D))  # 0.125

# bf16-space Schraudolph: bits16 = round(x*A16 + B16); bitcast -> ~exp(x/8)
SCH16_A = float(2**7 / np.log(2) * 0.125)
SCH16_B = float(127 * 2**7 - 5.0)
MASK_BIG = 40000.0  # saturates the i16 convert -> -32768 -> bf16 -0.0

# routing knobs
DIAG_DVE_KB_MIN = 6   # kb >= this: whole first chunk via DVE masked STT
N_FILLER_DVE = 0      # extra full 1024 chunks per head routed to DVE


def _patch_tile_drain():
    """This walrus build rejects >1 sem wait on the kernel-tail Drain
    instruction ("Too many sync wait commands"). Spread the waits across
    single-wait NOPs on the sync engine instead."""
    if getattr(tile.TileContext, "_drain_patched", False):
        return

    def _drain_and_barrier(self, tick_clock, wait_clock):
        gc = tick_clock.global_clock
        n = len(gc)
        for i in range(n):
            if gc[i] > 0:
                vc = VectorClock([gc[j] if j == i else 0 for j in range(n)])
                nop_inst = self.nc.sync.nop(nofuse=True, hint=f"drainwait{i}")
                wait_clock.add_sem_waits(nop_inst.ins, ScopedClock({None: vc}))
        self.nc.sync.drain()
        self.nc.all_engine_barrier()
        popped = self.nc._tile_sem_poison_stack.pop()
        assert popped is self._sem_poison
        self.nc.clear_and_free_semaphores(list(self.sems.allocated().values()))
        self.nc.all_engine_barrier()

    tile.TileContext._drain_and_barrier = _drain_and_barrier
    tile.TileContext._drain_patched = True


_patch_tile_drain()


def _split_multi_waits(nc, limit=1):
    """This walrus build allows at most one sem wait per instruction.
    Move excess waits onto same-engine NOPs inserted just before."""
    ctr = [0]
    for func in nc.m.functions:
        for bb in func.blocks:
            insts = list(bb.instructions)
            out = []
            changed = False
            for inst in insts:
                si = inst.sync_info
                if si is not None and si.on_wait is not None and len(si.on_wait) > limit:
                    waits = list(si.on_wait)
                    extra, keep = waits[:-limit], waits[-limit:]
                    for w in extra:
                        ctr[0] += 1
                        nop = mybir.InstNoOp(
                            name=f"waitsplit-{ctr[0]}", ins=[], outs=[]
                        )
                        nop.engine = inst.engine
                        nop.sync_info = mybir.SyncInfo(on_wait=[w], on_update=[])
                        out.append(nop)
                    inst.sync_info = mybir.SyncInfo(
                        on_wait=keep, on_update=list(si.on_update or [])
                    )
                    changed = True
                out.append(inst)
            if changed:
                try:
                    bb.instructions[:] = out
                except Exception:
                    bb.instructions = out
    return nc


DVE_KBS = {3, 5, 7, 9, 11, 13, 15}  # whole kb via DVE (interleaved w/ scalar)
EXTRA_DVE = {(1, 1024)}  # extra (kb, off) plain-DVE chunks for balance


def _chunk_plan():
    """Per kb: list of (off, CL, engine) with engine in {"dve_mask",
    "dve", "act"}; plus per-kb flag for post-exp trimask. DVE and scalar
    kbs alternate so both exp engines run concurrently."""
    plan = []
    for kb in range(NB):
        L = S - kb * 128
        chunks = []
        off = 0
        while off < L:
            cl = min(1024, L - off)
            if kb in DVE_KBS:
                eng = "dve_mask" if off == 0 else "dve"
            elif (kb, off) in EXTRA_DVE:
                eng = "dve"
            else:
                eng = "act"
            chunks.append((off, cl, eng))
            off += cl
        need_trimask = kb not in DVE_KBS
        plan.append((kb, L, chunks, need_trimask))
    return plan


def build_nc(n_heads: int = HEADS_PER_CORE):
    nc = bass.Bass("TRN2", target_bir_lowering=False)
    q_d = nc.dram_tensor("queries", [n_heads, S, D], F32, kind="ExternalInput")
    k_d = nc.dram_tensor("keys", [n_heads, S, D], F32, kind="ExternalInput")
    v_d = nc.dram_tensor("values", [n_heads, S, D], F32, kind="ExternalInput")
    o_d = nc.dram_tensor("out", [n_heads, S, D], F32, kind="ExternalOutput")

    v_r = v_d[:].rearrange("h (n p) d -> h p n d", p=128)
    o_r = o_d[:].rearrange("h (n p) d -> h p n d", p=128)

    plan = _chunk_plan()

    with tile.TileContext(nc) as tc:
        with (
            tc.tile_pool(name="const", bufs=1) as constp,
            tc.tile_pool(name="scr", bufs=8, space="DRAM") as scrp,
            tc.tile_pool(name="tp", bufs=4) as tpp,
            tc.tile_pool(name="vpool", bufs=4) as vpp,
            tc.tile_pool(name="ut", bufs=3) as utp,
            tc.tile_pool(name="oh", bufs=3) as ohp,
            tc.tile_pool(name="rz", bufs=4) as rzp,
            tc.tile_pool(name="ps_s", bufs=3, space="PSUM") as ps_s,
            tc.tile_pool(name="ps_o", bufs=2, space="PSUM") as ps_o,
        ):
            trimask = constp.tile([128, 128], BF16)
            make_upper_triangular(nc, trimask, val=1.0, diag=True)
            # STT mask for DVE diag chunks: -B16 on keep, +MASK_BIG on kill
            tri_f = constp.tile([128, 128], F32)
            make_upper_triangular(nc, tri_f, val=1.0, diag=True)
            zpad = constp.tile([128, 1024], BF16)
            nc.vector.memset(zpad, 0.0)
            mstt = constp.tile([128, 1024], F32)
            nc.vector.memset(mstt, -SCH16_B)
            nc.vector.tensor_scalar(
                out=mstt[:, 0:128], in0=tri_f, scalar1=-(MASK_BIG + SCH16_B),
                scalar2=MASK_BIG, op0=mybir.AluOpType.mult,
                op1=mybir.AluOpType.add,
            )

            # warm the ACT exp table at t=0 (overlaps the ~2.7us load)
            warm = constp.tile([128, 1], F32)
            nc.scalar.activation(
                out=warm, in_=tri_f[:, 0:1],
                func=mybir.ActivationFunctionType.Exp, scale=1.0,
            )

            PIPE = 3
            scrs = {}
            vps = {}
            loaded = {}

            def issue_casts(h):
                # [S, 128] scratch. Only K needs zeros in cols 64:128 (the
                # transposed K^T rows 64:127 are WEIGHT rows: zeros there
                # nullify whatever garbage sits in qt rows 64:127, so scrq
                # cols 64:128 can stay uninitialized). Early heads' DMAs are
                # split into row slices to spread descriptor generation
                # across queues (cuts the pipeline-fill latency).
                scrq = scrp.tile([S, 2 * D], BF16, tag="scrq")
                scrk = scrp.tile([S, 2 * D], BF16, tag="scrk")
                nsplit = 4 if h < 2 else 1
                step = S // nsplit
                for r0 in range(0, S, step):
                    nc.gpsimd.dma_start(
                        out=scrq[r0 : r0 + step, 0:D], in_=q_d[h, r0 : r0 + step]
                    )
                    nc.gpsimd.dma_start(
                        out=scrk[r0 : r0 + step, 0:D], in_=k_d[h, r0 : r0 + step]
                    )
                    nc.gpsimd.dma_start(
                        out=scrk[r0 : r0 + step, D : 2 * D],
                        in_=zpad[:, 0 : step * D // 128],
                    )
                scrs[h] = (scrq, scrk)

            def issue_v(h):
                vp = vpp.tile([128, NB, D + 1], BF16, tag="vp")
                nc.gpsimd.dma_start(out=vp[:, :, 0:D], in_=v_r[h])
                vps[h] = vp

            def issue_xbar(h):
                scrq, scrk = scrs.pop(h)
                qt = tpp.tile([128, S], BF16, tag="qt")
                kt = tpp.tile([128, S], BF16, tag="kt")
                nc.sync.dma_start(out=qt, in_=scrq, transpose=True)
                nc.sync.dma_start(out=kt, in_=scrk, transpose=True)
                loaded[h] = (qt, kt)

            for h in range(min(PIPE, n_heads)):
                issue_v(h)
            for h in range(n_heads):
                issue_casts(h)
            for h in range(min(PIPE, n_heads)):
                issue_xbar(h)

            def emit_pv_group(pv, gi):
                uts_p, vp_p, oh_p, h_p = pv
                g0, g1 = ((0, 7), (7, 14), (14, 16))[gi]
                cnt = g1 - g0
                po = ps_o.tile([128, 455], F32, tag="po")
                for qb in range(g0, g1):
                    j = qb - g0
                    for kb in range(qb + 1):
                        nc.tensor.matmul(
                            po[:, j * 65 : j * 65 + 65],
                            lhsT=uts_p[kb][:, (qb - kb) * 128 : (qb - kb) * 128 + 128],
                            rhs=vp_p[:, kb, :],
                            start=(kb == 0),
                            stop=(kb == qb),
                        )
                grp = po.rearrange("p (q c) -> p q c", c=65)
                rz = rzp.tile([128, 7], F32, tag="rz")
                nc.vector.reciprocal(rz[:, 0:cnt], grp[:, 0:cnt, 64:65])
                nc.vector.tensor_tensor(
                    out=oh_p[:, g0:g1, :],
                    in0=grp[:, 0:cnt, 0:64],
                    in1=rz[:, 0:cnt].unsqueeze(-1).broadcast_to((128, cnt, 64)),
                    op=mybir.AluOpType.mult,
                )
                if gi == 2:
                    nc.sync.dma_start(out=o_r[h_p], in_=oh_p)

            pv_prev = None
            for h in range(n_heads):
                if h + PIPE < n_heads:
                    issue_v(h + PIPE)
                    issue_xbar(h + PIPE)
                qt, kt = loaded.pop(h)
                vp = vps.pop(h)
                nc.vector.memset(vp[:, :, D : D + 1], 1.0)

                uts = []
                for kb, L, chunks, need_trimask in plan:
                    qs = kb * 128
                    ut = utp.tile([128, L], BF16, tag=f"ut{kb}")
                    uts.append(ut)
                    for off, cl, eng in chunks:
                        ps = ps_s.tile([128, 1024], F32, tag="s")
                        for c0 in range(0, cl, 512):
                            ce = min(512, cl - c0)
                            nc.tensor.matmul(
                                ps[:, c0 : c0 + ce],
                                lhsT=kt[:, qs : qs + 128],
                                rhs=qt[:, qs + off + c0 : qs + off + c0 + ce],
                                start=True,
                                stop=True,
                            )
                        if eng == "act":
                            nc.scalar.activation(
                                out=ut[:, off : off + cl],
                                in_=ps[:, 0:cl],
                                func=mybir.ActivationFunctionType.Exp,
                                scale=float(SCALE),
                            )
                        elif eng == "dve_mask":
                            nc.vector.scalar_tensor_tensor(
                                out=ut.bitcast(I16)[:, off : off + cl],
                                in0=ps[:, 0:cl],
                                scalar=SCH16_A,
                                in1=mstt[:, 0:cl],
                                op0=mybir.AluOpType.mult,
                                op1=mybir.AluOpType.subtract,
                            )
                        else:  # dve plain
                            nc.vector.tensor_scalar(
                                out=ut.bitcast(I16)[:, off : off + cl],
                                in0=ps[:, 0:cl],
                                scalar1=SCH16_A,
                                scalar2=SCH16_B,
                                op0=mybir.AluOpType.mult,
                                op1=mybir.AluOpType.add,
                            )
                    if need_trimask:
                        nc.vector.tensor_mul(ut[:, 0:128], ut[:, 0:128], trimask)
                oh = ohp.tile([128, NB, D], F32, tag="oh")
                pv_prev = (uts, vp, oh, h)
                for gi in range(3):
                    emit_pv_group(pv_prev, gi)
    _split_multi_waits(nc)
    return nc


_NC_CACHE = {}


def _get_nc(n_heads: int = HEADS_PER_CORE):
    if n_heads not in _NC_CACHE:
        _NC_CACHE[n_heads] = build_nc(n_heads)
    return _NC_CACHE[n_heads]


def make_in_maps(queries, keys, values):
    qf = np.ascontiguousarray(
        np.asarray(queries, dtype=np.float32).reshape(B * H, S, D)
    )
    kf = np.ascontiguousarray(np.asarray(keys, dtype=np.float32).reshape(B * H, S, D))
    vf = np.ascontiguousarray(
        np.asarray(values, dtype=np.float32).reshape(B * H, S, D)
    )
    n = HEADS_PER_CORE
    return [
        {
            "queries": qf[i * n : (i + 1) * n],
            "keys": kf[i * n : (i + 1) * n],
            "values": vf[i * n : (i + 1) * n],
        }
        for i in range(N_CORES)
    ]


def kernel(keys, queries, values, head_dim=None, **_ignored):
    nc = _get_nc()
    in_maps = make_in_maps(queries, keys, values)
    res = run_bass_kernel_spmd(nc, in_maps, core_ids=list(range(N_CORES)))
    out = np.concatenate([res.results[i]["out"] for i in range(N_CORES)], axis=0)
    return out.reshape(B, H, S, D).astype(np.float32)
